# revision 7
# baseline (speedup 1.0000x reference)
"""Masked multi-head attention (B=32, N=512, E=512, H=8) on 8 Trainium2 cores.

Sharding: data-parallel over batch (4 batches per core); weights and the
attention mask are replicated.  All layout transforms (weight transposes,
x transpose, mask transpose/cast, bias broadcast) are done host-side in
numpy so the device kernel is pure matmul/softmax work.

Per-core pipeline (per batch):
  qT = WqT.T @ xT (+bq)         [e_out, n]  e-major   (ScalarE bias add)
  kT = WkT.T @ xT (+bk)         [e_out, n]
  v  = xT.T @ WvT (+bv)         [n, e_out]  n-major, with a ones column
                                appended per head ([v_h | 1]) so the
                                softmax row-sum falls out of the P@V matmul
  per head h:
    sT[k,q] = kT_h.T @ qT_h     4 matmuls -> 4 PSUM banks [128, 2048]
    P = exp(sT / 8)             one ACTIVATE over all 4 banks (scale fused)
    P *= adjT                   VectorE mask multiply
    o[q, 0:65] = sum_k P_T[k,q].T @ [v_h | 1]   (col 64 = row-sum)
    o[:, 0:64] *= 1/o[:, 64]    reciprocal + tensor_scalar (per-q scale)
  oT = transpose(o)             PE transpose, 128x128 tiles
  out = oT.T @ WoT + bo         final projection, DMA to HBM
"""

import numpy as np

import concourse.bass as bass
import concourse.tile as tile
from concourse import bacc, mybir
import concourse.bass_utils as bass_utils
from concourse.masks import make_identity

N_CORES = 8
B, N, E, H = 32, 512, 512, 8
DH = E // H  # 64
BPC = B // N_CORES  # batches per core
P = 128
NT = N // P  # 4 tiles along sequence
ET = E // P  # 4 tiles along embedding
FP32 = mybir.dt.float32
AF = mybir.ActivationFunctionType


def build_nc(loop_iters=1):
    nc = bacc.Bacc("TRN2", target_bir_lowering=False, debug=False,
                   num_devices=N_CORES)

    xT_d = nc.dram_tensor("xT", [BPC, E, N], FP32, kind="ExternalInput")
    wq_d = nc.dram_tensor("WqT", [E, E], FP32, kind="ExternalInput")
    wk_d = nc.dram_tensor("WkT", [E, E], FP32, kind="ExternalInput")
    wv_d = nc.dram_tensor("WvT", [E, E], FP32, kind="ExternalInput")
    wo_d = nc.dram_tensor("WoT", [E, E], FP32, kind="ExternalInput")
    bq_d = nc.dram_tensor("bqT", [P, ET], FP32, kind="ExternalInput")
    bk_d = nc.dram_tensor("bkT", [P, ET], FP32, kind="ExternalInput")
    bv_d = nc.dram_tensor("bvB", [P, E], FP32, kind="ExternalInput")
    bo_d = nc.dram_tensor("boB", [P, E], FP32, kind="ExternalInput")
    adj_d = nc.dram_tensor("adjT", [N, N], FP32, kind="ExternalInput")
    out_d = nc.dram_tensor("out", [BPC, N, E], FP32, kind="ExternalOutput")

    with tile.TileContext(nc) as tc:
        with (
            tc.tile_pool(name="persist", bufs=1) as persist,
            tc.tile_pool(name="xt", bufs=2) as xt_pool,
            tc.tile_pool(name="qt", bufs=2) as qt_pool,
            tc.tile_pool(name="kt", bufs=2) as kt_pool,
            tc.tile_pool(name="vx", bufs=2) as vx_pool,
            tc.tile_pool(name="pt", bufs=2) as pt_pool,
            tc.tile_pool(name="osb", bufs=2) as o_pool,
            tc.tile_pool(name="otsb", bufs=2) as ot_pool,
            tc.tile_pool(name="outsb", bufs=3) as out_pool,
            tc.tile_pool(name="small", bufs=8) as small_pool,
            tc.tile_pool(name="ps_big", bufs=1, space="PSUM") as ps_big,
            tc.tile_pool(name="ps_small", bufs=4, space="PSUM") as ps_small,
        ):
            # ---- persistent tensors (replicated weights / mask / biases)
            wq_sb = persist.tile([P, ET, E], FP32)
            nc.sync.dma_start(wq_sb[:], wq_d.ap().rearrange("(c p) e -> p c e", p=P))
            wk_sb = persist.tile([P, ET, E], FP32)
            nc.sync.dma_start(wk_sb[:], wk_d.ap().rearrange("(c p) e -> p c e", p=P))
            wv_sb = persist.tile([P, ET, E], FP32)
            nc.sync.dma_start(wv_sb[:], wv_d.ap().rearrange("(c p) e -> p c e", p=P))
            wo_sb = persist.tile([P, ET, E], FP32)
            nc.sync.dma_start(wo_sb[:], wo_d.ap().rearrange("(c p) e -> p c e", p=P))
            adj_sb = persist.tile([P, NT, N], FP32)
            nc.sync.dma_start(adj_sb[:], adj_d.ap().rearrange("(c p) q -> p c q", p=P))
            bq_sb = persist.tile([P, ET], FP32)
            nc.sync.dma_start(bq_sb[:], bq_d.ap())
            bk_sb = persist.tile([P, ET], FP32)
            nc.sync.dma_start(bk_sb[:], bk_d.ap())
            bv_sb = persist.tile([P, E], FP32)
            nc.sync.dma_start(bv_sb[:], bv_d.ap())
            bo_sb = persist.tile([P, E], FP32)
            nc.sync.dma_start(bo_sb[:], bo_d.ap())
            ident = persist.tile([P, P], FP32)
            make_identity(nc, ident[:])

            import contextlib
            loop_cm = (tc.For_i(0, loop_iters, 1) if loop_iters > 1
                       else contextlib.nullcontext())
            with loop_cm:
                body(nc, tc, locals())

    nc.compile()
    return nc


def body(nc, tc, env):
    (xT_d, out_d, wq_sb, wk_sb, wv_sb, wo_sb, adj_sb, bq_sb, bk_sb, bv_sb,
     bo_sb, ident) = (env[k] for k in (
         "xT_d", "out_d", "wq_sb", "wk_sb", "wv_sb", "wo_sb", "adj_sb",
         "bq_sb", "bk_sb", "bv_sb", "bo_sb", "ident"))
    (xt_pool, qt_pool, kt_pool, vx_pool, pt_pool, o_pool, ot_pool, out_pool,
     small_pool, ps_big, ps_small) = (env[k] for k in (
         "xt_pool", "qt_pool", "kt_pool", "vx_pool", "pt_pool", "o_pool",
         "ot_pool", "out_pool", "small_pool", "ps_big", "ps_small"))
    if True:
            for b in range(BPC):
                xt = xt_pool.tile([P, ET, N], FP32)
                nc.sync.dma_start(
                    xt[:], xT_d.ap()[b].rearrange("(c p) n -> p c n", p=P))

                # ---- q/k projections, e-major output (qT[e_out, n])
                qt = qt_pool.tile([P, ET, N], FP32)
                ktl = kt_pool.tile([P, ET, N], FP32)
                for w_sb, b_sb, dst in ((wq_sb, bq_sb, qt), (wk_sb, bk_sb, ktl)):
                    for t in range(ET):
                        ps = ps_small.tile([P, N], FP32, tag="ps")
                        for kc in range(ET):
                            nc.tensor.matmul(
                                ps[:], w_sb[:, kc, t * P:(t + 1) * P],
                                xt[:, kc, :],
                                start=(kc == 0), stop=(kc == ET - 1))
                        nc.scalar.activation(
                            dst[:, t, :], ps[:], AF.Identity,
                            bias=b_sb[:, t:t + 1], scale=1.0)

                # ---- v projection, n-major ([n, (h, d)]) + ones column
                vx = vx_pool.tile([P, NT, H, DH + 1], FP32)
                nc.vector.memset(vx[:, :, :, DH:DH + 1], 1.0)
                for nt in range(NT):
                    ps = ps_small.tile([P, E], FP32, tag="ps")
                    for kc in range(ET):
                        nc.tensor.matmul(
                            ps[:], xt[:, kc, nt * P:(nt + 1) * P],
                            wv_sb[:, kc, :],
                            start=(kc == 0), stop=(kc == ET - 1))
                    nc.vector.tensor_add(
                        vx[:, nt, :, 0:DH],
                        ps.rearrange("p (h d) -> p h d", h=H),
                        bv_sb.rearrange("p (h d) -> p h d", h=H))

                # ---- attention per head
                o_sb = o_pool.tile([P, NT, E], FP32)
                for h in range(H):
                    t, po = h // 2, (h % 2) * DH
                    ps_s = ps_big.tile([P, NT * N], FP32, tag="scores")
                    for kt in range(NT):
                        nc.tensor.matmul(
                            ps_s[:, kt * N:(kt + 1) * N],
                            ktl[po:po + DH, t, kt * P:(kt + 1) * P],
                            qt[po:po + DH, t, :],
                            start=True, stop=True)
                    pt = pt_pool.tile([P, NT * N], FP32)
                    nc.scalar.activation(pt[:], ps_s[:], AF.Exp, scale=0.125)
                    nc.vector.tensor_mul(pt[:], pt[:],
                                         adj_sb.rearrange("p c q -> p (c q)"))
                    for qi in range(NT):
                        ps_o = ps_small.tile([P, DH + 1], FP32, tag="ps")
                        for kt in range(NT):
                            nc.tensor.matmul(
                                ps_o[:],
                                pt[:, kt * N + qi * P: kt * N + qi * P + P],
                                vx[:, kt, h, :],
                                start=(kt == 0), stop=(kt == NT - 1))
                        rc = small_pool.tile([P, 1], FP32, tag="rc")
                        nc.vector.reciprocal(rc[:], ps_o[:, DH:DH + 1])
                        nc.vector.tensor_scalar_mul(
                            o_sb[:, qi, h * DH:(h + 1) * DH],
                            ps_o[:, 0:DH], rc[:])

                # ---- transpose o -> oT (e-major) for the output projection
                ot = ot_pool.tile([P, ET, N], FP32)
                for et in range(ET):
                    for nt in range(NT):
                        ps_t = ps_small.tile([P, P], FP32, tag="ps")
                        nc.tensor.transpose(
                            ps_t[:], o_sb[:, nt, et * P:(et + 1) * P], ident[:])
                        nc.vector.tensor_copy(
                            ot[:, et, nt * P:(nt + 1) * P], ps_t[:])

                # ---- output projection + bias, store
                for nt in range(NT):
                    ps_f = ps_small.tile([P, E], FP32, tag="ps")
                    for et in range(ET):
                        nc.tensor.matmul(
                            ps_f[:], ot[:, et, nt * P:(nt + 1) * P],
                            wo_sb[:, et, :],
                            start=(et == 0), stop=(et == ET - 1))
                    ob = out_pool.tile([P, E], FP32)
                    nc.vector.tensor_add(ob[:], ps_f[:], bo_sb[:])
                    nc.sync.dma_start(out_d.ap()[b, nt * P:(nt + 1) * P, :], ob[:])


_NC_CACHE = {}


def get_nc(loop_iters=1):
    if loop_iters not in _NC_CACHE:
        _NC_CACHE[loop_iters] = build_nc(loop_iters)
    return _NC_CACHE[loop_iters]


def prep_inputs(x, adj, Wq, Wk, Wv, bq, bk, bv, Wo, bo):
    """Host-side layout prep -> per-core input maps."""
    x = np.asarray(x, dtype=np.float32)
    shared = {
        "WqT": np.ascontiguousarray(np.asarray(Wq, np.float32).T),
        "WkT": np.ascontiguousarray(np.asarray(Wk, np.float32).T),
        "WvT": np.ascontiguousarray(np.asarray(Wv, np.float32).T),
        "WoT": np.ascontiguousarray(np.asarray(Wo, np.float32).T),
        "bqT": np.ascontiguousarray(np.asarray(bq, np.float32).reshape(ET, P).T),
        "bkT": np.ascontiguousarray(np.asarray(bk, np.float32).reshape(ET, P).T),
        "bvB": np.ascontiguousarray(
            np.broadcast_to(np.asarray(bv, np.float32), (P, E))),
        "boB": np.ascontiguousarray(
            np.broadcast_to(np.asarray(bo, np.float32), (P, E))),
        "adjT": np.ascontiguousarray(np.asarray(adj).T.astype(np.float32)),
    }
    in_maps = []
    for c in range(N_CORES):
        xs = x[c * BPC:(c + 1) * BPC]  # [BPC, N, E]
        m = dict(shared)
        m["xT"] = np.ascontiguousarray(xs.transpose(0, 2, 1))
        in_maps.append(m)
    return in_maps


def kernel(**inputs):
    nc = get_nc()
    in_maps = prep_inputs(**inputs)
    res = bass_utils.run_bass_kernel_spmd(
        nc, in_maps, core_ids=list(range(N_CORES)))
    return np.concatenate([r["out"] for r in res.results], axis=0)


# ---------------------------------------------------------------------------
# Benchmarking helpers (not used by the grading path). Runs the kernel with
# inputs resident on device, with the whole per-core computation repeated
# R times inside the NEFF (tc.For_i); HW time per iteration is estimated as
# (T(R2) - T(R1)) / (R2 - R1) to cancel the fixed dispatch overhead.
def _make_sharded_fn(nc):
    import jax
    from jax.sharding import Mesh, PartitionSpec, NamedSharding
    from jax.experimental.shard_map import shard_map
    from concourse import bass2jax

    bass2jax.install_neuronx_cc_hook()
    pid = nc.partition_id_tensor
    in_names, out_names, out_avals = [], [], []
    for alloc in nc.m.functions[0].allocations:
        if not isinstance(alloc, mybir.MemoryLocationSet):
            continue
        name = alloc.memorylocations[0].name
        if alloc.kind == "ExternalInput":
            if pid is None or name != pid.name:
                in_names.append(name)
        elif alloc.kind == "ExternalOutput":
            out_names.append(name)
            out_avals.append(jax.core.ShapedArray(
                tuple(alloc.tensor_shape), mybir.dt.np(alloc.dtype)))
    all_in_names = in_names + out_names
    if pid is not None:
        all_in_names.append(pid.name)

    def _body(*args):
        operands = list(args)
        if pid is not None:
            operands.append(bass2jax.partition_id_tensor())
        return tuple(bass2jax._bass_exec_p.bind(
            *operands,
            out_avals=tuple(out_avals),
            in_names=tuple(all_in_names),
            out_names=tuple(out_names),
            lowering_input_output_aliases=(),
            sim_require_finite=True,
            sim_require_nnan=True,
            nc=nc,
        ))

    devices = jax.devices()[:N_CORES]
    mesh = Mesh(np.asarray(devices), ("core",))
    spec = PartitionSpec("core")
    nin = len(in_names) + len(out_names)
    fn = jax.jit(
        shard_map(_body, mesh=mesh, in_specs=(spec,) * nin,
                  out_specs=(spec,) * len(out_names), check_rep=False),
        keep_unused=True,
    )
    return fn, in_names, out_names, out_avals, mesh, spec


def _time_nc(nc, in_maps, n_rep):
    import time
    import jax
    from jax.sharding import NamedSharding

    fn, in_names, out_names, out_avals, mesh, spec = _make_sharded_fn(nc)
    sh = NamedSharding(mesh, spec)
    args = []
    for name in in_names:
        args.append(jax.device_put(
            np.concatenate([m[name] for m in in_maps], axis=0), sh))
    for av in out_avals:
        args.append(jax.device_put(
            np.zeros((N_CORES * av.shape[0],) + av.shape[1:], av.dtype), sh))
    out = fn(*args)
    jax.block_until_ready(out)
    ts = []
    for _ in range(n_rep):
        t0 = time.perf_counter()
        out = fn(*args)
        jax.block_until_ready(out)
        ts.append(time.perf_counter() - t0)
    return min(ts), out


def benchmark(inputs, r1=2, r2=18, n_rep=8):
    in_maps = prep_inputs(**inputs)
    t1, _ = _time_nc(get_nc(r1), in_maps, n_rep)
    t2, _ = _time_nc(get_nc(r2), in_maps, n_rep)
    return (t2 - t1) * 1e9 / (r2 - r1)


# revision 24
# speedup vs baseline: 23.5242x; 23.5242x over previous
"""Masked multi-head attention (B=32, N=512, E=512, H=8) on 8 Trainium2 cores.

Sharding: data-parallel over batch (4 batches per core); weights and the
attention mask are replicated.  All layout transforms (weight transposes,
x transpose, mask transpose/cast, bias broadcast) are done host-side in
numpy so the device kernel is pure matmul/softmax work.

Per-core pipeline (per batch):
  qT = WqT.T @ xT (+bq)         [e_out, n]  e-major   (ScalarE bias add)
  kT = WkT.T @ xT (+bk)         [e_out, n]
  v  = xT.T @ WvT (+bv)         [n, e_out]  n-major, with a ones column
                                appended per head ([v_h | 1]) so the
                                softmax row-sum falls out of the P@V matmul
  per head h:
    sT[k,q] = kT_h.T @ qT_h     4 matmuls -> 4 PSUM banks [128, 2048]
    P = exp(sT / 8)             one ACTIVATE over all 4 banks (scale fused)
    P *= adjT                   VectorE mask multiply
    o[q, 0:65] = sum_k P_T[k,q].T @ [v_h | 1]   (col 64 = row-sum)
    o[:, 0:64] *= 1/o[:, 64]    reciprocal + tensor_scalar (per-q scale)
  oT = transpose(o)             PE transpose, 128x128 tiles
  out = oT.T @ WoT + bo         final projection, DMA to HBM
"""

import numpy as np

import concourse.bass as bass
import concourse.tile as tile
from concourse import bacc, mybir
import concourse.bass_utils as bass_utils
from concourse.masks import make_identity

N_CORES = 8
B, N, E, H = 32, 512, 512, 8
DH = E // H  # 64
BPC = B // N_CORES  # batches per core
P = 128
NT = N // P  # 4 tiles along sequence
ET = E // P  # 4 tiles along embedding
FP32 = mybir.dt.float32
BF16 = mybir.dt.bfloat16
AF = mybir.ActivationFunctionType


def _mm(ap):
    return ap


def build_nc(loop_iters=1):
    nc = bacc.Bacc("TRN2", target_bir_lowering=False, debug=False,
                   num_devices=N_CORES)

    xT_d = nc.dram_tensor("xT", [BPC, E, N], BF16, kind="ExternalInput")
    wq_d = nc.dram_tensor("WqT", [E, E], BF16, kind="ExternalInput")
    wk_d = nc.dram_tensor("WkT", [E, E], BF16, kind="ExternalInput")
    wv_d = nc.dram_tensor("WvT", [E, E], BF16, kind="ExternalInput")
    wo_d = nc.dram_tensor("WoT", [E, E], BF16, kind="ExternalInput")
    bq_d = nc.dram_tensor("bqT", [P, ET], FP32, kind="ExternalInput")
    bk_d = nc.dram_tensor("bkT", [P, ET], FP32, kind="ExternalInput")
    bv_d = nc.dram_tensor("bvB", [P, E], FP32, kind="ExternalInput")
    bo_d = nc.dram_tensor("boB", [P, E], FP32, kind="ExternalInput")
    adj_d = nc.dram_tensor("adjT", [N, N], BF16, kind="ExternalInput")
    out_d = nc.dram_tensor("out", [BPC, N, E], FP32, kind="ExternalOutput")

    with tile.TileContext(nc) as tc:
        with (
            tc.tile_pool(name="persist", bufs=1) as persist,
            tc.tile_pool(name="xt", bufs=2) as xt_pool,
            tc.tile_pool(name="qt", bufs=2) as qt_pool,
            tc.tile_pool(name="kt", bufs=2) as kt_pool,
            tc.tile_pool(name="vx", bufs=2) as vx_pool,
            tc.tile_pool(name="pt", bufs=3) as pt_pool,
            tc.tile_pool(name="osb", bufs=2) as o_pool,
            tc.tile_pool(name="otsb", bufs=2) as ot_pool,
            tc.tile_pool(name="outsb", bufs=3) as out_pool,
            tc.tile_pool(name="small", bufs=8) as small_pool,
            tc.tile_pool(name="ps_big", bufs=2, space="PSUM") as ps_big,
            tc.tile_pool(name="ps_small", bufs=4, space="PSUM") as ps_small,
        ):
            # ---- persistent tensors (replicated weights / mask / biases)
            wq_sb = persist.tile([P, ET, E], BF16)
            nc.sync.dma_start(wq_sb[:], wq_d.ap().rearrange("(c p) e -> p c e", p=P))
            wk_sb = persist.tile([P, ET, E], BF16)
            nc.sync.dma_start(wk_sb[:], wk_d.ap().rearrange("(c p) e -> p c e", p=P))
            wv_sb = persist.tile([P, ET, E], BF16)
            nc.sync.dma_start(wv_sb[:], wv_d.ap().rearrange("(c p) e -> p c e", p=P))
            wo_sb = persist.tile([P, ET, E], BF16)
            nc.sync.dma_start(wo_sb[:], wo_d.ap().rearrange("(c p) e -> p c e", p=P))
            adj_sb = persist.tile([P, NT, N], BF16)
            nc.sync.dma_start(adj_sb[:], adj_d.ap().rearrange("(c p) q -> p c q", p=P))
            bq_sb = persist.tile([P, ET], FP32)
            nc.sync.dma_start(bq_sb[:], bq_d.ap())
            bk_sb = persist.tile([P, ET], FP32)
            nc.sync.dma_start(bk_sb[:], bk_d.ap())
            bv_sb = persist.tile([P, E], FP32)
            nc.sync.dma_start(bv_sb[:], bv_d.ap())
            bo_sb = persist.tile([P, E], FP32)
            nc.sync.dma_start(bo_sb[:], bo_d.ap())
            ident = persist.tile([P, P], BF16)
            make_identity(nc, ident[:])

            import contextlib
            loop_cm = (tc.For_i(0, loop_iters, 1) if loop_iters > 1
                       else contextlib.nullcontext())
            with loop_cm:
                body(nc, tc, locals())

    nc.compile()
    return nc


def body(nc, tc, env):
    (xT_d, out_d, wq_sb, wk_sb, wv_sb, wo_sb, adj_sb, bq_sb, bk_sb, bv_sb,
     bo_sb, ident) = (env[k] for k in (
         "xT_d", "out_d", "wq_sb", "wk_sb", "wv_sb", "wo_sb", "adj_sb",
         "bq_sb", "bk_sb", "bv_sb", "bo_sb", "ident"))
    (xt_pool, qt_pool, kt_pool, vx_pool, pt_pool, o_pool, ot_pool, out_pool,
     small_pool, ps_big, ps_small) = (env[k] for k in (
         "xt_pool", "qt_pool", "kt_pool", "vx_pool", "pt_pool", "o_pool",
         "ot_pool", "out_pool", "small_pool", "ps_big", "ps_small"))
    if True:
            for b in range(BPC):
                xt = xt_pool.tile([P, ET, N], BF16)
                nc.sync.dma_start(
                    xt[:], xT_d.ap()[b].rearrange("(c p) n -> p c n", p=P))

                # ---- q/k projections, e-major output (qT[e_out, n])
                qt = qt_pool.tile([P, ET, N], BF16)
                ktl = kt_pool.tile([P, ET, N], BF16)
                for w_sb, b_sb, dst in ((wq_sb, bq_sb, qt), (wk_sb, bk_sb, ktl)):
                    for t in range(ET):
                        ps = ps_small.tile([P, N], FP32, tag="ps")
                        for kc in range(ET):
                            nc.tensor.matmul(
                                ps[:], _mm(w_sb[:, kc, t * P:(t + 1) * P]),
                                _mm(xt[:, kc, :]),
                                start=(kc == 0), stop=(kc == ET - 1))
                        nc.scalar.activation(
                            dst[:, t, :], ps[:], AF.Identity,
                            bias=b_sb[:, t:t + 1], scale=1.0)

                # ---- v projection, n-major ([n, (h, d)]) + ones column
                vx = vx_pool.tile([P, NT, H, DH + 1], BF16)
                nc.vector.memset(vx[:, :, :, DH:DH + 1], 1.0)
                for nt in range(NT):
                    ps = ps_small.tile([P, E], FP32, tag="ps")
                    for kc in range(ET):
                        nc.tensor.matmul(
                            ps[:], _mm(xt[:, kc, nt * P:(nt + 1) * P]),
                            _mm(wv_sb[:, kc, :]),
                            start=(kc == 0), stop=(kc == ET - 1))
                    nc.vector.tensor_add(
                        vx[:, nt, :, 0:DH],
                        ps.rearrange("p (h d) -> p h d", h=H),
                        bv_sb.rearrange("p (h d) -> p h d", h=H))

                # ---- attention per head
                o_sb = o_pool.tile([P, NT, E], BF16)
                adj_flat = adj_sb.rearrange("p c q -> p (c q)")
                for h in range(H):
                    t, po = h // 2, (h % 2) * DH
                    pt = pt_pool.tile([P, NT * N], BF16)
                    for half in range(2):
                        ps_s = ps_big.tile([P, 2 * N], FP32, tag="scores")
                        for k2 in range(2):
                            kt = half * 2 + k2
                            nc.tensor.matmul(
                                ps_s[:, k2 * N:(k2 + 1) * N],
                                _mm(ktl[po:po + DH, t, kt * P:(kt + 1) * P]),
                                _mm(qt[po:po + DH, t, :]),
                                start=True, stop=True)
                        sl = slice(half * 2 * N, (half + 1) * 2 * N)
                        nc.scalar.activation(pt[:, sl], ps_s[:], AF.Exp,
                                             scale=0.125)
                        nc.vector.tensor_mul(pt[:, sl], pt[:, sl],
                                             adj_flat[:, sl])
                    for qi in range(NT):
                        ps_o = ps_small.tile([P, DH + 1], FP32, tag="ps")
                        for kt in range(NT):
                            nc.tensor.matmul(
                                ps_o[:],
                                _mm(pt[:, kt * N + qi * P: kt * N + qi * P + P]),
                                _mm(vx[:, kt, h, :]),
                                start=(kt == 0), stop=(kt == NT - 1))
                        rc = small_pool.tile([P, 1], FP32, tag="rc")
                        nc.vector.reciprocal(rc[:], ps_o[:, DH:DH + 1])
                        nc.vector.tensor_scalar_mul(
                            o_sb[:, qi, h * DH:(h + 1) * DH],
                            ps_o[:, 0:DH], rc[:])

                # ---- transpose o -> oT (e-major) for the output projection
                ot = ot_pool.tile([P, ET, N], BF16)
                for et in range(ET):
                    for nt in range(NT):
                        ps_t = ps_small.tile([P, P], BF16, tag="ps")
                        nc.tensor.transpose(
                            ps_t[:], o_sb[:, nt, et * P:(et + 1) * P], ident[:])
                        nc.vector.tensor_copy(
                            ot[:, et, nt * P:(nt + 1) * P], ps_t[:])

                # ---- output projection + bias, store
                for nt in range(NT):
                    ps_f = ps_small.tile([P, E], FP32, tag="ps")
                    for et in range(ET):
                        nc.tensor.matmul(
                            ps_f[:], _mm(ot[:, et, nt * P:(nt + 1) * P]),
                            _mm(wo_sb[:, et, :]),
                            start=(et == 0), stop=(et == ET - 1))
                    ob = out_pool.tile([P, E], FP32)
                    nc.vector.tensor_add(ob[:], ps_f[:], bo_sb[:])
                    nc.sync.dma_start(out_d.ap()[b, nt * P:(nt + 1) * P, :], ob[:])


_NC_CACHE = {}


def get_nc(loop_iters=1):
    if loop_iters not in _NC_CACHE:
        _NC_CACHE[loop_iters] = build_nc(loop_iters)
    return _NC_CACHE[loop_iters]


def prep_inputs(x, adj, Wq, Wk, Wv, bq, bk, bv, Wo, bo):
    """Host-side layout prep -> per-core input maps."""
    import ml_dtypes  # noqa: F401 (used below)
    x = np.asarray(x, dtype=np.float32)
    import ml_dtypes
    shared = {
        "WqT": np.ascontiguousarray(np.asarray(Wq, np.float32).T.astype(ml_dtypes.bfloat16)),
        "WkT": np.ascontiguousarray(np.asarray(Wk, np.float32).T.astype(ml_dtypes.bfloat16)),
        "WvT": np.ascontiguousarray(np.asarray(Wv, np.float32).T.astype(ml_dtypes.bfloat16)),
        "WoT": np.ascontiguousarray(np.asarray(Wo, np.float32).T.astype(ml_dtypes.bfloat16)),
        "bqT": np.ascontiguousarray(np.asarray(bq, np.float32).reshape(ET, P).T),
        "bkT": np.ascontiguousarray(np.asarray(bk, np.float32).reshape(ET, P).T),
        "bvB": np.ascontiguousarray(
            np.broadcast_to(np.asarray(bv, np.float32), (P, E))),
        "boB": np.ascontiguousarray(
            np.broadcast_to(np.asarray(bo, np.float32), (P, E))),
        "adjT": np.ascontiguousarray(
            np.asarray(adj).T.astype(ml_dtypes.bfloat16)),
    }
    in_maps = []
    for c in range(N_CORES):
        xs = x[c * BPC:(c + 1) * BPC]  # [BPC, N, E]
        m = dict(shared)
        m["xT"] = np.ascontiguousarray(
            xs.transpose(0, 2, 1).astype(ml_dtypes.bfloat16))
        in_maps.append(m)
    return in_maps


def kernel(**inputs):
    nc = get_nc()
    in_maps = prep_inputs(**inputs)
    res = bass_utils.run_bass_kernel_spmd(
        nc, in_maps, core_ids=list(range(N_CORES)))
    return np.concatenate([r["out"] for r in res.results], axis=0)


# ---------------------------------------------------------------------------
# Benchmarking helpers (not used by the grading path). Runs the kernel with
# inputs resident on device, with the whole per-core computation repeated
# R times inside the NEFF (tc.For_i); HW time per iteration is estimated as
# (T(R2) - T(R1)) / (R2 - R1) to cancel the fixed dispatch overhead.
def _make_sharded_fn(nc):
    import jax
    from jax.sharding import Mesh, PartitionSpec, NamedSharding
    from jax.experimental.shard_map import shard_map
    from concourse import bass2jax

    bass2jax.install_neuronx_cc_hook()
    pid = nc.partition_id_tensor
    in_names, out_names, out_avals = [], [], []
    for alloc in nc.m.functions[0].allocations:
        if not isinstance(alloc, mybir.MemoryLocationSet):
            continue
        name = alloc.memorylocations[0].name
        if alloc.kind == "ExternalInput":
            if pid is None or name != pid.name:
                in_names.append(name)
        elif alloc.kind == "ExternalOutput":
            out_names.append(name)
            out_avals.append(jax.core.ShapedArray(
                tuple(alloc.tensor_shape), mybir.dt.np(alloc.dtype)))
    all_in_names = in_names + out_names
    if pid is not None:
        all_in_names.append(pid.name)

    def _body(*args):
        operands = list(args)
        if pid is not None:
            operands.append(bass2jax.partition_id_tensor())
        return tuple(bass2jax._bass_exec_p.bind(
            *operands,
            out_avals=tuple(out_avals),
            in_names=tuple(all_in_names),
            out_names=tuple(out_names),
            lowering_input_output_aliases=(),
            sim_require_finite=True,
            sim_require_nnan=True,
            nc=nc,
        ))

    devices = jax.devices()[:N_CORES]
    mesh = Mesh(np.asarray(devices), ("core",))
    spec = PartitionSpec("core")
    nin = len(in_names) + len(out_names)
    fn = jax.jit(
        shard_map(_body, mesh=mesh, in_specs=(spec,) * nin,
                  out_specs=(spec,) * len(out_names), check_rep=False),
        keep_unused=True,
    )
    return fn, in_names, out_names, out_avals, mesh, spec


def _time_nc(nc, in_maps, n_rep):
    import time
    import jax
    from jax.sharding import NamedSharding

    fn, in_names, out_names, out_avals, mesh, spec = _make_sharded_fn(nc)
    sh = NamedSharding(mesh, spec)
    args = []
    for name in in_names:
        args.append(jax.device_put(
            np.concatenate([m[name] for m in in_maps], axis=0), sh))
    for av in out_avals:
        args.append(jax.device_put(
            np.zeros((N_CORES * av.shape[0],) + av.shape[1:], av.dtype), sh))
    out = fn(*args)
    jax.block_until_ready(out)
    ts = []
    for _ in range(n_rep):
        t0 = time.perf_counter()
        out = fn(*args)
        jax.block_until_ready(out)
        ts.append(time.perf_counter() - t0)
    return min(ts), out


def benchmark(inputs, r1=2, r2=18, n_rep=8):
    in_maps = prep_inputs(**inputs)
    t1, _ = _time_nc(get_nc(r1), in_maps, n_rep)
    t2, _ = _time_nc(get_nc(r2), in_maps, n_rep)
    return (t2 - t1) * 1e9 / (r2 - r1)
